# revision 2
# baseline (speedup 1.0000x reference)
"""GRU-D-T kernel, data-parallel over batch across 8 NeuronCores.

Sharding: batch B=1024 split 128/core across the 8 cores (axis 0 of
x/dt/mask/tp); all weights replicated; the scan carry (hidden state)
stays with its batch shard. Inputs are taken full-size, outputs are
gathered back to full shape on the host.
"""

import numpy as np

B, T, I, H = 1024, 100, 128, 256
NCORES = 8

_BATCH_KEYS = ("x", "dt", "mask", "tp")
_WEIGHT_KEYS = (
    "x_mean", "h0", "w_gx", "h_gx", "w_gh", "h_gh", "w_t", "h_t",
    "W_ih", "W_hh", "b_ih", "b_hh", "W1", "b1", "W2", "b2",
)

_COMPILED = {}


def _model(x, dt, mask, tp, x_mean, h0, w_gx, h_gx, w_gh, h_gh, w_t, h_t,
           W_ih, W_hh, b_ih, b_hh, W1, b1, W2, b2):
    import jax, jax.numpy as jnp

    Bs = x.shape[0]
    h_init = jnp.broadcast_to(h0, (Bs, H))
    x_prev = jnp.concatenate([jnp.zeros_like(x[:, :1]), x[:, :-1]], axis=1)
    is_first = (jnp.arange(T) == 0)
    W_ihT = W_ih.T
    W_hhT = W_hh.T
    b_g = b_ih + b_hh

    def step(h, inputs):
        xt, xp, dtt, mt, tpt, first = inputs
        gamma_x = jnp.exp(-jnp.maximum(0.0, dtt @ w_gx + h_gx))
        gamma_x = jnp.where(first, 0.0, gamma_x)
        gamma_h = jnp.exp(-jnp.maximum(0.0, dtt @ w_gh + h_gh))
        hs = gamma_h * h
        xs = xt * mt + (1.0 - mt) * (gamma_x * xp + (1.0 - gamma_x) * x_mean)
        gt = jax.nn.sigmoid(tpt @ w_t + h_t)
        gi = xs @ W_ihT + b_ih
        gh = hs @ W_hhT + b_hh
        i_r, i_z, i_n = jnp.split(gi, 3, axis=-1)
        h_r, h_z, h_n = jnp.split(gh, 3, axis=-1)
        r = jax.nn.sigmoid(i_r + h_r)
        z = jax.nn.sigmoid(i_z + h_z)
        n = jnp.tanh(i_n + r * h_n)
        h1 = gt * ((1.0 - z) * n + z * hs)
        hid = jax.nn.sigmoid(h1 @ W1.T + b1)
        logits = jax.nn.sigmoid(hid @ W2.T + b2)
        out = jax.nn.softmax(logits, axis=1)
        return h1, (out, xs)

    seq = (jnp.swapaxes(x, 0, 1), jnp.swapaxes(x_prev, 0, 1),
           jnp.swapaxes(dt, 0, 1), jnp.swapaxes(mask, 0, 1),
           jnp.swapaxes(tp, 0, 1), is_first)
    _, (outputs, xhat) = jax.lax.scan(step, h_init, seq)
    return outputs, xhat


def _get_fn():
    import jax
    if "fn" in _COMPILED:
        return _COMPILED["fn"]
    devs = jax.devices()[:NCORES]
    fn = jax.pmap(
        _model,
        in_axes=(0, 0, 0, 0) + (None,) * len(_WEIGHT_KEYS),
        devices=devs,
    )
    _COMPILED["fn"] = fn
    return fn


def kernel(**inputs):
    fn = _get_fn()
    shard = inputs["x"].shape[0] // NCORES
    batch_args = [
        np.ascontiguousarray(
            np.asarray(inputs[k]).reshape(NCORES, shard, *inputs[k].shape[1:])
        )
        for k in _BATCH_KEYS
    ]
    weight_args = [np.asarray(inputs[k]) for k in _WEIGHT_KEYS]
    outs, xhat = fn(*batch_args, *weight_args)
    outs = np.asarray(outs)   # [8, T, shard, 2]
    xhat = np.asarray(xhat)   # [8, T, shard, I]
    outs_full = np.concatenate(list(outs), axis=1)
    xhat_full = np.concatenate(list(xhat), axis=1)
    return outs_full, xhat_full
